# revision 43
# baseline (speedup 1.0000x reference)
"""LocationSensitiveSoftAttention on 8 Trainium2 NeuronCores (Bass/Tile), v3.

Contract: kernel(**inputs) takes the FULL unsharded inputs (numpy arrays, keys
as in setup_inputs()) and returns the FULL output [64, 1, 256] fp32.

Strategy: data-parallel over batch B=64 -> 8 batches per core; weights
replicated. Math restructure (exact up to fp rounding):
  pre[b,t,:] = memory[b,t,:] @ (Wm@We) + sum_k spad[b,t+k] * G[k,:] + r[b,:]
     G[k,u]  = sum_c conv_w[c,0,k] * (Wl@We)[c,u]     (conv folded into loc)
     r[b,:]  = (query[b,1]@Wq + bq + bm + bl) @ We + be + conv_b @ (Wl@We)
  h = tanh(pre); energy = h @ v_a; s = sigmoid(energy)
  w = state + s/sum(s)
  context = (w @ memory) @ Wm + (sum(state) + 1) * bm

Precision plan (numerically validated, rel-err ~1.2e-2 < 2e-2 gate):
  - pre-GEMM path (memory, WmWe, shifts, G, v_a, h): fp8e4m3. The alignment
    s/sum(s) contributes ~0.1% of the context magnitude, so fp8 noise there is
    invisible. The memory GEMM runs in DoubleRow mode (2 fp8 MACs/cell).
  - context reduction (w @ memory): memory in fp8e3m4 (4 mantissa bits),
    w stationary in bf16 (mixed-dtype matmul, exact on HW).
  - final @ Wm in fp32.
Memory is read once per layout: e-major fp8e4 (1MB/batch) for the GEMM and
t-major fp8e3 (1MB/batch) for the context reduction, vs 4MB/batch before.
"""

import sys

for _p in ("/root/.axon_site", "/root/.axon_site/_ro/trn_rl_repo",
           "/root/.axon_site/_ro/pypackages", "/opt/trn_rl_repo"):
    if _p not in sys.path:
        sys.path.append(_p)

import numpy as np
import ml_dtypes

B, TQ, T = 64, 2, 2048
HID, ENC, U, FILT, K = 1024, 512, 256, 32, 31
N_CORES = 8
PB = B // N_CORES  # batches per core
PAD = K // 2  # 15
NT = T // 128  # 16 t-tiles
NBLK = T // 512  # 4 t-blocks

BF16 = ml_dtypes.bfloat16
E4 = ml_dtypes.float8_e4m3
E3 = ml_dtypes.float8_e3m4

_BUILT = {}
TRACE = False
LAST_RESULTS = None
MARKERS = []


def _build_nc(repeat=1):
    import concourse.bacc as bacc
    import concourse.mybir as mybir
    import concourse.tile as tile
    import concourse.bass as bass

    f32 = mybir.dt.float32
    bf16 = mybir.dt.bfloat16
    e4 = mybir.dt.float8e4
    e3 = mybir.dt.float8e3
    AF = mybir.ActivationFunctionType
    ALU = mybir.AluOpType
    AX = mybir.AxisListType
    DR = mybir.MatmulPerfMode.DoubleRow

    nc = bacc.Bacc("TRN2", target_bir_lowering=False, debug=False,
                   num_devices=N_CORES)

    # ---- DRAM I/O ----
    # memE[b, p, kc, j, t] = fp8e4(mem[b, t, kc*256 + j*128 + p])
    meme_d = nc.dram_tensor("meme", [PB, 128, 2, 2, T], e4, kind="ExternalInput")
    # memT[b, p, ti, e] = fp8e3(mem[b, ti*128 + p, e])
    memt_d = nc.dram_tensor("memt", [PB, 128, NT, ENC], e3, kind="ExternalInput")
    # shifted8[b, p, j, t] = fp8e4(spad[b, t + j*16 + p]), k = j*16+p < 31;
    # row (15, 1) = 0. DoubleRow layout, Ki=16.
    sh_d = nc.dram_tensor("sh8", [PB, 16, 2, T], e4, kind="ExternalInput")
    # All constants packed into three per-dtype blobs (HWDGE issue
    # overhead is ~630ns per DMA, so 17 loads -> 4). blob8 is split into a
    # critical part (GEMM weights) and the prologue-only wqwe tail.
    # blob8 cols: [0:1024] wmwe8[kc][j][u], [1024:1152] q1t[kc][j][b16],
    #   [1152:1408] va8[tb][j][m32], [1408:1920] g8[j][u] (rows 0-15),
    #   [1920:3968] wqwe8[kc][j][u].
    # blobb cols: [0:128] idb, [128:384] c0 (row 0), [384:385] mask4
    #   (rows 0-3 = 1.0), [385:1409] wm[ec][u] (bf16).
    # blobf cols: [0:128] stateT[b][ti], [128:256] idf, [256:512] bm (row 0).
    blob8_d = nc.dram_tensor("blob8", [128, 3968], e4, kind="ExternalInput")
    blobb_d = nc.dram_tensor("blobb", [128, 1409], bf16, kind="ExternalInput")
    blobf_d = nc.dram_tensor("blobf", [128, 512], f32, kind="ExternalInput")
    out_d = nc.dram_tensor("out", [PB, U], f32, kind="ExternalOutput")

    with tile.TileContext(nc) as tc:
        with (
            tc.tile_pool(name="consts", bufs=1) as consts,
            tc.tile_pool(name="meme", bufs=5) as memep,
            tc.tile_pool(name="memt", bufs=5) as memtp,
            tc.tile_pool(name="shp", bufs=5) as shp,
            tc.tile_pool(name="hdr", bufs=3) as hdrp,
            tc.tile_pool(name="srow", bufs=2) as srowp,
            tc.tile_pool(name="psPre", bufs=3, space="PSUM") as psPre,
            tc.tile_pool(name="psEn", bufs=2, space="PSUM") as psEn,
            tc.tile_pool(name="psT", bufs=2, space="PSUM") as psT,
            tc.tile_pool(name="psM", bufs=1, space="PSUM") as psM,
        ):
          def _body():
            def MARK(label):
                names = set()
                for blk in nc.m.functions[0].blocks:
                    for inst in blk.instructions:
                        names.add(inst.name)
                MARKERS.append((label, names))
            MARK("consts")
            # ---- ACT table warmup: the Tanh/Sigmoid/Copy table loads
            # (~1.3us each) happen at first use; fire them on dummy data
            # while the first DMAs are still in flight ----
            warm = consts.tile([1, 8], f32, tag="warm")
            nc.vector.memset(warm[:], 0.0)
            warm2 = consts.tile([1, 8], bf16, tag="warm2")
            nc.scalar.activation(warm2[:], warm[:], AF.Tanh)
            nc.scalar.activation(warm2[:], warm[:], AF.Sigmoid)
            nc.scalar.activation(warm2[:], warm[:], AF.Copy)
            # ---- constants: 3 blob DMAs; fp8+bf16 first (gate prologue
            # and GEMM), f32 (w path + epilogue) after batch-0/1 loads ----
            blob8 = consts.tile([128, 3968], e4, tag="blob8")
            nc.scalar.dma_start(out=blob8[:, 0:1920], in_=blob8_d.ap()[:, 0:1920])
            blobb = consts.tile([128, 1409], bf16, tag="blobb")
            nc.scalar.dma_start(out=blobb[:], in_=blobb_d.ap())
            nc.scalar.dma_start(out=blob8[:, 1920:3968],
                                in_=blob8_d.ap()[:, 1920:3968])
            idb_sb = blobb[:, 0:128]
            c0_sb = blobb[0:1, 128:384]
            mask4 = blobb[0:32, 384:385]
            wm_sb = [blobb[:, 385 + ec * 256:385 + (ec + 1) * 256]
                     for ec in range(4)]

            # DoubleRow operand APs must be built explicitly: tile slicing
            # emits APs whose dim-1 (the k-tile axis the DR mode keys on) is
            # wrong, which compiles to silently-incorrect matmuls.
            def dr3(tile_, offset_elems, jstep, n, nparts=None):
                base = tile_[:]
                p0 = base.ap[0] if nparts is None else [base.ap[0][0], nparts]
                return bass.AP(tensor=base.tensor,
                               offset=base.offset + offset_elems,
                               ap=[p0, [jstep, 2], [1, n]])
            OF_WMWE, OF_Q1T, OF_VA, OF_G, OF_WQWE = 0, 1024, 1152, 1408, 1920

            # ---- r = q1 @ (Wq@We) + c0 -> rT [128, PB] per vch (tanh bias).
            # 4 DR matmuls + 2 PE transposes; no DMA transposes, so r is
            # ready ~1.5us in and never gates the tanh pipeline. ----
            ones8 = consts.tile([1, 8], bf16, tag="ones8")
            nc.vector.memset(ones8[:], 1.0)
            r_ps = psM.tile([16, U], f32, tag="misc")
            for kc in range(4):
                nc.tensor.matmul(r_ps[:],
                                 dr3(blob8, OF_Q1T + kc * 32, 16, 16),
                                 dr3(blob8, OF_WQWE + kc * 512, U, U),
                                 start=(kc == 0), stop=False, perf_mode=DR)
            nc.tensor.matmul(r_ps[0:PB, :], ones8[:], c0_sb[:],
                             start=False, stop=True, skip_group_check=True)
            r_bf = consts.tile([PB, U], bf16, tag="rbf")
            nc.scalar.activation(r_bf[:], r_ps[0:PB, :], AF.Copy)
            rT_sb = []
            for vch in range(2):
                tp = psM.tile([128, PB], bf16, tag="misc")
                nc.tensor.matmul(tp[:], r_bf[:, vch * 128:(vch + 1) * 128],
                                 idb_sb[0:PB, 0:PB], is_transpose=True)
                t_ = consts.tile([128, PB], f32, tag=f"rT{vch}")
                nc.vector.tensor_copy(t_[:], tp[:])
                rT_sb.append(t_)

            # ---- first two batches' GEMM inputs (batch 0 chunked by
            # t-block so its first matmuls start ~2us earlier) ----
            def load_a(b, chunked=False):
                meme = memep.tile([128, 2, 2, T], e4, tag="meme", name=f"meme{b}")
                if chunked:
                    for tb in range(NBLK):
                        nc.sync.dma_start(
                            out=meme[:, :, :, tb * 512:(tb + 1) * 512],
                            in_=meme_d.ap()[b][:, :, :, tb * 512:(tb + 1) * 512])
                else:
                    nc.sync.dma_start(out=meme[:], in_=meme_d.ap()[b])
                sh = shp.tile([16, 2, T], e4, tag="sh", name=f"sh{b}")
                nc.sync.dma_start(out=sh[:], in_=sh_d.ap()[b])
                return {"meme": meme, "sh": sh}

            sts = {}
            sts[0] = load_a(0, chunked=True)
            sts[1] = load_a(1)

            # ---- f32 constants (w path + epilogue; needed from stage_b(0)) ----
            blobf = consts.tile([128, 512], f32, tag="blobf")
            nc.scalar.dma_start(out=blobf[:], in_=blobf_d.ap())
            statet_sb = blobf[:, 0:128]
            idf_sb = blobf[:, 128:256]
            bm_sb = blobf[0:1, 256:512]
            ones128b = consts.tile([1, 128], bf16, tag="ones128b")
            nc.vector.memset(ones128b[:], 1.0)
            ones128f = consts.tile([128, 1], f32, tag="ones128f")
            nc.vector.memset(ones128f[:], 1.0)
            callT_sb = []
            for ch in range(4):
                t_ = consts.tile([128, PB], bf16, tag=f"callT{ch}")
                callT_sb.append(t_)

            # ---- sum(state) + 1 row [1, 8] ----
            red = consts.tile([128, PB], f32, tag="red")
            st3d = bass.AP(tensor=statet_sb.tensor, offset=statet_sb.offset,
                           ap=[statet_sb.ap[0], [NT, PB], [1, NT]])
            nc.vector.tensor_reduce(red[:], st3d, axis=AX.X, op=ALU.add)
            st_ps = psM.tile([1, PB], f32, tag="misc")
            nc.tensor.matmul(st_ps[:], ones128f[:], red[:], start=True, stop=True)
            sig_row = consts.tile([1, PB], f32, tag="sigrow")
            nc.vector.tensor_scalar_add(sig_row[:], st_ps[:], 1.0)

            # ---- per-batch pipeline ----
            def load_t(b, st):
                memt = memtp.tile([128, NT, ENC], e3, tag="memt", name=f"memt{b}")
                nc.sync.dma_start(out=memt[:], in_=memt_d.ap()[b])
                st["memt"] = memt

            def stage_a(b, st, prev_st=None):
                """GEMM + tanh + energy + sigmoid for batch b. The previous
                batch's rowsum matmul is emitted before the last t-block so
                its reciprocal chain resolves on DVE while PE finishes tb3."""
                meme, sh = st["meme"], st["sh"]
                en4 = psEn.tile([32, 512], f32, tag="en")
                for tb in range(NBLK):
                    if tb == NBLK - 1 and prev_st is not None:
                        emit_rowsum(prev_st)
                    h_dr = hdrp.tile([128, 2, 512], e4, tag="hdr")
                    for vch in range(2):
                        pre = psPre.tile([128, 512], f32, tag="pre")
                        for kc in range(2):
                            nc.tensor.matmul(
                                pre[:],
                                dr3(blob8, OF_WMWE + kc * 512 + vch * 128,
                                    U, 128),
                                dr3(meme, kc * 2 * T + tb * 512, T, 512),
                                start=(kc == 0), stop=False, perf_mode=DR)
                        nc.tensor.matmul(
                            pre[:],
                            dr3(blob8, OF_G + vch * 128, U, 128, nparts=16),
                            dr3(sh, tb * 512, T, 512),
                            start=False, stop=True, perf_mode=DR)
                        nc.scalar.activation(h_dr[:, vch, :], pre[:], AF.Tanh,
                                             bias=rT_sb[vch][:, b:b + 1])
                    nc.tensor.matmul(en4[:],
                                     dr3(blob8, OF_VA + tb * 64, 32, 32),
                                     dr3(h_dr, 0, 512, 512),
                                     start=(tb == 0), stop=(tb == NBLK - 1),
                                     perf_mode=DR)
                s4 = srowp.tile([32, 512], bf16, tag="s4", name=f"s4_{b}")
                nc.scalar.activation(s4[:], en4[:], AF.Sigmoid)
                st.update(s4=s4)

            def emit_rowsum(st):
                """sum(s) -> 1/sum(s) as bf16, kicked off early."""
                rowsum_ps = psM.tile([1, 512], f32, tag="misc")
                nc.tensor.matmul(rowsum_ps[:], mask4[:], st["s4"][:],
                                 start=True, stop=True)
                t2 = srowp.tile([1, 1], f32, tag="t2")
                nc.vector.tensor_reduce(t2[:], rowsum_ps[:], axis=AX.X,
                                        op=ALU.add)
                rec = srowp.tile([1, 1], f32, tag="rec")
                nc.vector.reciprocal(rec[:], t2[:])
                rec_bf = srowp.tile([1, 1], bf16, tag="recbf")
                nc.vector.tensor_copy(rec_bf[:], rec[:])
                st["rec_bf"] = rec_bf

            def stage_b(b, st):
                """w = state + s/sum(s); context vector for batch b."""
                s4, memt = st["s4"], st["memt"]
                if "rec_bf" not in st:
                    emit_rowsum(st)
                rec_bf = st["rec_bf"]
                recb_ps = psM.tile([128, 1], f32, tag="misc")
                nc.tensor.matmul(recb_ps[:], ones128b[:], rec_bf[:],
                                 start=True, stop=True)
                recb = srowp.tile([128, 1], f32, tag="recb")
                nc.vector.tensor_copy(recb[:], recb_ps[:])
                wT = srowp.tile([128, 4, 4], bf16, tag="wT", name=f"wT{b}")
                for c4 in range(4):
                    pst = psT.tile([128, 32], bf16, tag="pst")
                    nc.tensor.matmul(pst[:], s4[:, c4 * 128:(c4 + 1) * 128],
                                     idb_sb[0:32, 0:32], is_transpose=True)
                    nc.vector.scalar_tensor_tensor(
                        wT[:, c4, :], in0=pst[:, 0:4], scalar=recb[:],
                        in1=statet_sb[:, b * NT + c4 * 4:b * NT + (c4 + 1) * 4],
                        op0=ALU.mult, op1=ALU.add)
                cv_ps = psM.tile([1, ENC], f32, tag="misc")
                for ti in range(NT):
                    nc.tensor.matmul(cv_ps[:], wT[:, ti % 4, ti // 4:ti // 4 + 1],
                                     memt[:, ti, :],
                                     start=(ti == 0), stop=(ti == NT - 1))
                cv_sb = srowp.tile([1, ENC], f32, tag="cvsb")
                nc.vector.tensor_copy(cv_sb[:], cv_ps[:])
                for ch in range(4):
                    tp = psT.tile([128, 1], f32, tag="pst", name=f"cvT{b}_{ch}")
                    nc.tensor.matmul(tp[:], cv_sb[0:1, ch * 128:(ch + 1) * 128],
                                     idf_sb[0:1, 0:1], is_transpose=True)
                    nc.vector.tensor_copy(callT_sb[ch][:, b:b + 1], tp[:])

            load_t(0, sts[0])
            load_t(1, sts[1])
            for b in range(PB):
                if b + 2 < PB and (b + 2) not in sts:
                    sts[b + 2] = load_a(b + 2)
                    load_t(b + 2, sts[b + 2])
                if b == 0 and b + 3 < PB:
                    sts[b + 3] = load_a(b + 3)
                    load_t(b + 3, sts[b + 3])
                MARK(f"a{b}")
                stage_a(b, sts[b], prev_st=sts.get(b - 1))
                if b >= 1:
                    MARK(f"b{b-1}")
                    stage_b(b - 1, sts[b - 1])
                    del sts[b - 1]
            MARK(f"b{PB-1}")
            stage_b(PB - 1, sts[PB - 1])
            MARK("epilogue")

            # ---- final: context = Call @ Wm + sig_row^T * bm ----
            ctx_ps = psM.tile([PB, U], f32, tag="misc")
            for ch in range(4):
                nc.tensor.matmul(ctx_ps[:], callT_sb[ch][:], wm_sb[ch][:],
                                 start=(ch == 0), stop=False)
            nc.tensor.matmul(ctx_ps[:], sig_row[:], bm_sb[:],
                             start=False, stop=True)
            ctx_sb = consts.tile([PB, U], f32, tag="ctx")
            nc.vector.tensor_copy(ctx_sb[:], ctx_ps[:])
            nc.sync.dma_start(out=out_d.ap(), in_=ctx_sb[:])

          for _rep in range(repeat):
              _body()
    nc.compile()
    return nc


def _host_prep(inputs):
    """Fold weights on host (weight-only transforms) and shard per core."""
    f32 = np.float32
    Wq = np.asarray(inputs["Wq"], f32)
    bq = np.asarray(inputs["bq"], f32)
    Wm = np.asarray(inputs["Wm"], f32)
    bm = np.asarray(inputs["bm"], f32)
    Wl = np.asarray(inputs["Wl"], f32)
    bl = np.asarray(inputs["bl"], f32)
    conv_w = np.asarray(inputs["conv_w"], f32)
    conv_b = np.asarray(inputs["conv_b"], f32)
    We = np.asarray(inputs["We"], f32)
    be = np.asarray(inputs["be"], f32)
    v_a = np.asarray(inputs["v_a"], f32)

    WmWe = (Wm @ We).astype(f32)
    WqWe = (Wq @ We).astype(f32)
    WlWe = (Wl @ We).astype(f32)
    G = np.einsum("ck,cu->ku", conv_w[:, 0, :], WlWe).astype(f32)
    c0 = ((bq + bm + bl) @ We + be + conv_b @ WlWe).astype(f32)

    query = np.asarray(inputs["query"], f32)
    state = np.asarray(inputs["state"], f32)
    memory = np.ascontiguousarray(np.asarray(inputs["memory"], f32))

    spad = np.zeros((B, T + 2 * PAD), f32)
    spad[:, PAD:PAD + T] = state
    q1 = np.ascontiguousarray(query[:, 1, :])

    ident = np.eye(128, dtype=f32)
    # wmwe8[kc, p, j, u] = WmWe[kc*256 + j*128 + p, u]
    wmwe8 = np.ascontiguousarray(
        WmWe.astype(E4).reshape(2, 2, 128, U).transpose(0, 2, 1, 3))
    wqwe8 = np.ascontiguousarray(
        WqWe.astype(E4).reshape(4, 2, 128, U).transpose(0, 2, 1, 3))
    g8 = np.zeros((32, U), E4)
    g8[:K] = G.astype(E4)
    g8 = np.ascontiguousarray(g8.reshape(2, 16, U).transpose(1, 0, 2))
    va8 = np.zeros((128, 4, 2, 32), E4)
    vat = v_a.astype(E4).reshape(2, 128).transpose(1, 0)  # [p, j]
    for tb in range(4):
        va8[:, tb, :, tb] = vat
    # blobb: idb | c0 row | mask4 col (rows 0-3) | wm — shared
    blobb = np.zeros((128, 1409), BF16)
    blobb[:, 0:128] = ident.astype(BF16)
    blobb[0, 128:384] = c0.astype(BF16)
    blobb[0:4, 384] = 1.0
    # per-partition flattened blob8 pieces (q1t added per core)
    wqwe_f = wqwe8.transpose(1, 0, 2, 3).reshape(128, 2048)
    wmwe_f = wmwe8.transpose(1, 0, 2, 3).reshape(128, 1024)
    va_f = va8.reshape(128, 256)
    g_f = np.zeros((128, 512), E4)
    g_f[0:16] = g8.reshape(16, 512)
    blobb[:, 385:1409] = (Wm.astype(BF16).reshape(4, 128, U)
                          .transpose(1, 0, 2).reshape(128, 1024))

    in_maps = []
    for c in range(N_CORES):
        sl = slice(c * PB, (c + 1) * PB)
        m = {"blobb": blobb}
        mb = memory[sl]  # [PB, T, ENC] f32
        # memE[b, p, kc, j, t] = e4(mem[b, t, kc*256 + j*128 + p])
        m8 = np.ascontiguousarray(mb.transpose(0, 2, 1)).astype(E4)  # [PB,ENC,T]
        m["meme"] = np.ascontiguousarray(
            m8.reshape(PB, 2, 2, 128, T).transpose(0, 3, 1, 2, 4))
        # memT[b, p, ti, e] = e3(mem[b, ti*128 + p, e])
        m["memt"] = np.ascontiguousarray(
            mb.astype(E3).reshape(PB, NT, 128, ENC).transpose(0, 2, 1, 3))
        # shifted8[b, p, j, t] = e4(spad[b, t + j*16 + p]); row (15,1) = 0
        sp8 = spad[sl].astype(E4)
        sh8 = np.zeros((PB, 32, T), E4)
        for k in range(K):
            sh8[:, k, :] = sp8[:, k:k + T]
        m["sh8"] = np.ascontiguousarray(
            sh8.reshape(PB, 2, 16, T).transpose(0, 2, 1, 3))
        # blob8: wqwe | wmwe | q1t | va | g
        q1t = np.zeros((128, 4, 2, 16), E4)
        q1t[:, :, :, 0:PB] = (q1[sl].astype(E4)
                              .reshape(PB, 4, 2, 128).transpose(3, 1, 2, 0))
        blob8 = np.zeros((128, 3968), E4)
        blob8[:, 0:1024] = wmwe_f
        blob8[:, 1024:1152] = q1t.reshape(128, 128)
        blob8[:, 1152:1408] = va_f
        blob8[:, 1408:1920] = g_f
        blob8[:, 1920:3968] = wqwe_f
        m["blob8"] = blob8
        # blobf: stateT | idf | bm row
        a = state[sl].reshape(PB, 4, 4, 128)  # [b, tb, c4, p]
        statet = a.transpose(3, 0, 2, 1).reshape(128, PB * NT)
        blobf = np.zeros((128, 512), f32)
        blobf[:, 0:128] = statet
        blobf[:, 128:256] = ident
        blobf[0, 256:512] = bm
        m["blobf"] = blobf
        in_maps.append(m)
    return in_maps


def kernel(**inputs) -> np.ndarray:
    global LAST_RESULTS
    from concourse import bass_utils

    if "nc" not in _BUILT:
        _BUILT["nc"] = _build_nc()
    nc = _BUILT["nc"]

    in_maps = _host_prep(inputs)
    res = bass_utils.run_bass_kernel_spmd(
        nc, in_maps, core_ids=list(range(N_CORES)), trace=TRACE)
    LAST_RESULTS = res
    out = np.concatenate([res.results[c]["out"] for c in range(N_CORES)], axis=0)
    return out.reshape(B, 1, U).astype(np.float32)


# revision 44
# speedup vs baseline: 752.4507x; 752.4507x over previous
"""LocationSensitiveSoftAttention on 8 Trainium2 NeuronCores (Bass/Tile), v3.

Contract: kernel(**inputs) takes the FULL unsharded inputs (numpy arrays, keys
as in setup_inputs()) and returns the FULL output [64, 1, 256] fp32.

Strategy: data-parallel over batch B=64 -> 8 batches per core; weights
replicated. Math restructure (exact up to fp rounding):
  pre[b,t,:] = memory[b,t,:] @ (Wm@We) + sum_k spad[b,t+k] * G[k,:] + r[b,:]
     G[k,u]  = sum_c conv_w[c,0,k] * (Wl@We)[c,u]     (conv folded into loc)
     r[b,:]  = (query[b,1]@Wq + bq + bm + bl) @ We + be + conv_b @ (Wl@We)
  h = tanh(pre); energy = h @ v_a; s = sigmoid(energy)
  w = state + s/sum(s)
  context = (w @ memory) @ Wm + (sum(state) + 1) * bm

Precision plan (numerically validated, rel-err ~1.2e-2 < 2e-2 gate):
  - pre-GEMM path (memory, WmWe, shifts, G, v_a, h): fp8e4m3. The alignment
    s/sum(s) contributes ~0.1% of the context magnitude, so fp8 noise there is
    invisible. The memory GEMM runs in DoubleRow mode (2 fp8 MACs/cell).
  - context reduction (w @ memory): memory in fp8e3m4 (4 mantissa bits),
    w stationary in bf16 (mixed-dtype matmul, exact on HW).
  - final @ Wm in fp32.
Memory is read once per layout: e-major fp8e4 (1MB/batch) for the GEMM and
t-major fp8e3 (1MB/batch) for the context reduction, vs 4MB/batch before.
"""

import sys

for _p in ("/root/.axon_site", "/root/.axon_site/_ro/trn_rl_repo",
           "/root/.axon_site/_ro/pypackages", "/opt/trn_rl_repo"):
    if _p not in sys.path:
        sys.path.append(_p)

import numpy as np
import ml_dtypes

B, TQ, T = 64, 2, 2048
HID, ENC, U, FILT, K = 1024, 512, 256, 32, 31
N_CORES = 8
PB = B // N_CORES  # batches per core
PAD = K // 2  # 15
NT = T // 128  # 16 t-tiles
NBLK = T // 512  # 4 t-blocks

BF16 = ml_dtypes.bfloat16
E4 = ml_dtypes.float8_e4m3
E3 = ml_dtypes.float8_e3m4

_BUILT = {}
TRACE = False
LAST_RESULTS = None
MARKERS = []


def _build_nc(repeat=1):
    import concourse.bacc as bacc
    import concourse.mybir as mybir
    import concourse.tile as tile
    import concourse.bass as bass

    f32 = mybir.dt.float32
    bf16 = mybir.dt.bfloat16
    e4 = mybir.dt.float8e4
    e3 = mybir.dt.float8e3
    AF = mybir.ActivationFunctionType
    ALU = mybir.AluOpType
    AX = mybir.AxisListType
    DR = mybir.MatmulPerfMode.DoubleRow

    nc = bacc.Bacc("TRN2", target_bir_lowering=False, debug=False,
                   num_devices=N_CORES)

    # ---- DRAM I/O ----
    # memE[b, p, kc, j, t] = fp8e4(mem[b, t, kc*256 + j*128 + p])
    meme_d = nc.dram_tensor("meme", [PB, 128, 2, 2, T], e4, kind="ExternalInput")
    # memT[b, p, ti, e] = fp8e3(mem[b, ti*128 + p, e])
    memt_d = nc.dram_tensor("memt", [PB, 128, NT, ENC], e3, kind="ExternalInput")
    # shifted8[b, p, j, t] = fp8e4(spad[b, t + j*16 + p]), k = j*16+p < 31;
    # row (15, 1) = 0. DoubleRow layout, Ki=16.
    sh_d = nc.dram_tensor("sh8", [PB, 16, 2, T], e4, kind="ExternalInput")
    # All constants packed into three per-dtype blobs (HWDGE issue
    # overhead is ~630ns per DMA, so 17 loads -> 4). blob8 is split into a
    # critical part (GEMM weights) and the prologue-only wqwe tail.
    # blob8 cols: [0:1024] wmwe8[kc][j][u], [1024:1152] q1t[kc][j][b16],
    #   [1152:1408] va8[tb][j][m32], [1408:1920] g8[j][u] (rows 0-15),
    #   [1920:3968] wqwe8[kc][j][u].
    # blobb cols: [0:128] idb, [128:384] c0 (row 0), [384:385] mask4
    #   (rows 0-3 = 1.0), [385:1409] wm[ec][u] (bf16).
    # blobf cols: [0:128] stateT[b][ti], [128:256] idf, [256:512] bm (row 0).
    blob8_d = nc.dram_tensor("blob8", [128, 3968], e4, kind="ExternalInput")
    blobb_d = nc.dram_tensor("blobb", [128, 1409], bf16, kind="ExternalInput")
    blobf_d = nc.dram_tensor("blobf", [128, 512], f32, kind="ExternalInput")
    out_d = nc.dram_tensor("out", [PB, U], f32, kind="ExternalOutput")

    with tile.TileContext(nc) as tc:
        with (
            tc.tile_pool(name="consts", bufs=1) as consts,
            tc.tile_pool(name="meme", bufs=5) as memep,
            tc.tile_pool(name="memt", bufs=5) as memtp,
            tc.tile_pool(name="shp", bufs=5) as shp,
            tc.tile_pool(name="hdr", bufs=3) as hdrp,
            tc.tile_pool(name="srow", bufs=2) as srowp,
            tc.tile_pool(name="psPre", bufs=3, space="PSUM") as psPre,
            tc.tile_pool(name="psEn", bufs=2, space="PSUM") as psEn,
            tc.tile_pool(name="psT", bufs=2, space="PSUM") as psT,
            tc.tile_pool(name="psM", bufs=1, space="PSUM") as psM,
        ):
          def _body():
            def MARK(label):
                names = set()
                for blk in nc.m.functions[0].blocks:
                    for inst in blk.instructions:
                        names.add(inst.name)
                MARKERS.append((label, names))
            MARK("consts")
            # ---- ACT table warmup: the Tanh/Sigmoid/Copy table loads
            # (~1.3us each) happen at first use; fire them on dummy data
            # while the first DMAs are still in flight ----
            warm = consts.tile([1, 8], f32, tag="warm")
            nc.vector.memset(warm[:], 0.0)
            warm2 = consts.tile([1, 8], bf16, tag="warm2")
            nc.scalar.activation(warm2[:], warm[:], AF.Tanh)
            nc.scalar.activation(warm2[:], warm[:], AF.Sigmoid)
            nc.scalar.activation(warm2[:], warm[:], AF.Copy)
            # ---- constants: 3 blob DMAs; fp8+bf16 first (gate prologue
            # and GEMM), f32 (w path + epilogue) after batch-0/1 loads ----
            blob8 = consts.tile([128, 3968], e4, tag="blob8")
            nc.scalar.dma_start(out=blob8[:, 0:1920], in_=blob8_d.ap()[:, 0:1920])
            blobb = consts.tile([128, 1409], bf16, tag="blobb")
            nc.scalar.dma_start(out=blobb[:], in_=blobb_d.ap())
            nc.scalar.dma_start(out=blob8[:, 1920:3968],
                                in_=blob8_d.ap()[:, 1920:3968])
            idb_sb = blobb[:, 0:128]
            c0_sb = blobb[0:1, 128:384]
            mask4 = blobb[0:32, 384:385]
            wm_sb = [blobb[:, 385 + ec * 256:385 + (ec + 1) * 256]
                     for ec in range(4)]

            # DoubleRow operand APs must be built explicitly: tile slicing
            # emits APs whose dim-1 (the k-tile axis the DR mode keys on) is
            # wrong, which compiles to silently-incorrect matmuls.
            def dr3(tile_, offset_elems, jstep, n, nparts=None):
                base = tile_[:]
                p0 = base.ap[0] if nparts is None else [base.ap[0][0], nparts]
                return bass.AP(tensor=base.tensor,
                               offset=base.offset + offset_elems,
                               ap=[p0, [jstep, 2], [1, n]])
            OF_WMWE, OF_Q1T, OF_VA, OF_G, OF_WQWE = 0, 1024, 1152, 1408, 1920

            # ---- PE clock warmup: the tensor engine starts throttled
            # (0.65-1.2 GHz) until ~3us of sustained activity. Burn dummy
            # matmuls while the first DMAs are in flight so batch 0 runs at
            # full clock. ----
            wsrc = consts.tile([1, 128], bf16, tag="wsrc")
            nc.vector.memset(wsrc[:], 0.0)
            wps = psT.tile([128, 128], f32, tag="pst", name="warmps")
            for i in range(20):
                nc.tensor.matmul(wps[:], wsrc[:], wsrc[:],
                                 start=(i == 0), stop=(i == 19))

            # ---- r = q1 @ (Wq@We) + c0 -> rT [128, PB] per vch (tanh bias).
            # 4 DR matmuls + 2 PE transposes; no DMA transposes, so r is
            # ready ~1.5us in and never gates the tanh pipeline. ----
            ones8 = consts.tile([1, 8], bf16, tag="ones8")
            nc.vector.memset(ones8[:], 1.0)
            r_ps = psM.tile([16, U], f32, tag="misc")
            for kc in range(4):
                nc.tensor.matmul(r_ps[:],
                                 dr3(blob8, OF_Q1T + kc * 32, 16, 16),
                                 dr3(blob8, OF_WQWE + kc * 512, U, U),
                                 start=(kc == 0), stop=False, perf_mode=DR)
            nc.tensor.matmul(r_ps[0:PB, :], ones8[:], c0_sb[:],
                             start=False, stop=True, skip_group_check=True)
            r_bf = consts.tile([PB, U], bf16, tag="rbf")
            nc.scalar.activation(r_bf[:], r_ps[0:PB, :], AF.Copy)
            rT_sb = []
            for vch in range(2):
                tp = psM.tile([128, PB], bf16, tag="misc")
                nc.tensor.matmul(tp[:], r_bf[:, vch * 128:(vch + 1) * 128],
                                 idb_sb[0:PB, 0:PB], is_transpose=True)
                t_ = consts.tile([128, PB], f32, tag=f"rT{vch}")
                nc.vector.tensor_copy(t_[:], tp[:])
                rT_sb.append(t_)

            # ---- first two batches' GEMM inputs (batch 0 chunked by
            # t-block so its first matmuls start ~2us earlier) ----
            def load_a(b, chunked=False):
                meme = memep.tile([128, 2, 2, T], e4, tag="meme", name=f"meme{b}")
                if chunked:
                    for tb in range(NBLK):
                        nc.sync.dma_start(
                            out=meme[:, :, :, tb * 512:(tb + 1) * 512],
                            in_=meme_d.ap()[b][:, :, :, tb * 512:(tb + 1) * 512])
                else:
                    nc.sync.dma_start(out=meme[:], in_=meme_d.ap()[b])
                sh = shp.tile([16, 2, T], e4, tag="sh", name=f"sh{b}")
                nc.sync.dma_start(out=sh[:], in_=sh_d.ap()[b])
                return {"meme": meme, "sh": sh}

            sts = {}
            sts[0] = load_a(0, chunked=True)
            sts[1] = load_a(1)

            # ---- f32 constants (w path + epilogue; needed from stage_b(0)) ----
            blobf = consts.tile([128, 512], f32, tag="blobf")
            nc.scalar.dma_start(out=blobf[:], in_=blobf_d.ap())
            statet_sb = blobf[:, 0:128]
            idf_sb = blobf[:, 128:256]
            bm_sb = blobf[0:1, 256:512]
            ones128b = consts.tile([1, 128], bf16, tag="ones128b")
            nc.vector.memset(ones128b[:], 1.0)
            ones128f = consts.tile([128, 1], f32, tag="ones128f")
            nc.vector.memset(ones128f[:], 1.0)
            callT_sb = []
            for ch in range(4):
                t_ = consts.tile([128, PB], bf16, tag=f"callT{ch}")
                callT_sb.append(t_)

            # ---- sum(state) + 1 row [1, 8] ----
            red = consts.tile([128, PB], f32, tag="red")
            st3d = bass.AP(tensor=statet_sb.tensor, offset=statet_sb.offset,
                           ap=[statet_sb.ap[0], [NT, PB], [1, NT]])
            nc.vector.tensor_reduce(red[:], st3d, axis=AX.X, op=ALU.add)
            st_ps = psM.tile([1, PB], f32, tag="misc")
            nc.tensor.matmul(st_ps[:], ones128f[:], red[:], start=True, stop=True)
            sig_row = consts.tile([1, PB], f32, tag="sigrow")
            nc.vector.tensor_scalar_add(sig_row[:], st_ps[:], 1.0)

            # ---- per-batch pipeline ----
            def load_t(b, st):
                memt = memtp.tile([128, NT, ENC], e3, tag="memt", name=f"memt{b}")
                nc.sync.dma_start(out=memt[:], in_=memt_d.ap()[b])
                st["memt"] = memt

            def stage_a(b, st, prev_st=None):
                """GEMM + tanh + energy + sigmoid for batch b. The previous
                batch's rowsum matmul is emitted before the last t-block so
                its reciprocal chain resolves on DVE while PE finishes tb3."""
                meme, sh = st["meme"], st["sh"]
                en4 = psEn.tile([32, 512], f32, tag="en")
                for tb in range(NBLK):
                    if tb == NBLK - 1 and prev_st is not None:
                        emit_rowsum(prev_st)
                    h_dr = hdrp.tile([128, 2, 512], e4, tag="hdr")
                    for vch in range(2):
                        pre = psPre.tile([128, 512], f32, tag="pre")
                        for kc in range(2):
                            nc.tensor.matmul(
                                pre[:],
                                dr3(blob8, OF_WMWE + kc * 512 + vch * 128,
                                    U, 128),
                                dr3(meme, kc * 2 * T + tb * 512, T, 512),
                                start=(kc == 0), stop=False, perf_mode=DR)
                        nc.tensor.matmul(
                            pre[:],
                            dr3(blob8, OF_G + vch * 128, U, 128, nparts=16),
                            dr3(sh, tb * 512, T, 512),
                            start=False, stop=True, perf_mode=DR)
                        nc.scalar.activation(h_dr[:, vch, :], pre[:], AF.Tanh,
                                             bias=rT_sb[vch][:, b:b + 1])
                    nc.tensor.matmul(en4[:],
                                     dr3(blob8, OF_VA + tb * 64, 32, 32),
                                     dr3(h_dr, 0, 512, 512),
                                     start=(tb == 0), stop=(tb == NBLK - 1),
                                     perf_mode=DR)
                s4 = srowp.tile([32, 512], bf16, tag="s4", name=f"s4_{b}")
                nc.scalar.activation(s4[:], en4[:], AF.Sigmoid)
                st.update(s4=s4)

            def emit_rowsum(st):
                """sum(s) -> 1/sum(s) as bf16, kicked off early."""
                rowsum_ps = psM.tile([1, 512], f32, tag="misc")
                nc.tensor.matmul(rowsum_ps[:], mask4[:], st["s4"][:],
                                 start=True, stop=True)
                t2 = srowp.tile([1, 1], f32, tag="t2")
                nc.vector.tensor_reduce(t2[:], rowsum_ps[:], axis=AX.X,
                                        op=ALU.add)
                rec = srowp.tile([1, 1], f32, tag="rec")
                nc.vector.reciprocal(rec[:], t2[:])
                rec_bf = srowp.tile([1, 1], bf16, tag="recbf")
                nc.vector.tensor_copy(rec_bf[:], rec[:])
                st["rec_bf"] = rec_bf

            def stage_b(b, st):
                """w = state + s/sum(s); context vector for batch b."""
                s4, memt = st["s4"], st["memt"]
                if "rec_bf" not in st:
                    emit_rowsum(st)
                rec_bf = st["rec_bf"]
                recb_ps = psM.tile([128, 1], f32, tag="misc")
                nc.tensor.matmul(recb_ps[:], ones128b[:], rec_bf[:],
                                 start=True, stop=True)
                recb = srowp.tile([128, 1], f32, tag="recb")
                nc.vector.tensor_copy(recb[:], recb_ps[:])
                wT = srowp.tile([128, 4, 4], bf16, tag="wT", name=f"wT{b}")
                for c4 in range(4):
                    pst = psT.tile([128, 32], bf16, tag="pst")
                    nc.tensor.matmul(pst[:], s4[:, c4 * 128:(c4 + 1) * 128],
                                     idb_sb[0:32, 0:32], is_transpose=True)
                    nc.vector.scalar_tensor_tensor(
                        wT[:, c4, :], in0=pst[:, 0:4], scalar=recb[:],
                        in1=statet_sb[:, b * NT + c4 * 4:b * NT + (c4 + 1) * 4],
                        op0=ALU.mult, op1=ALU.add)
                cv_ps = psM.tile([1, ENC], f32, tag="misc")
                for ti in range(NT):
                    nc.tensor.matmul(cv_ps[:], wT[:, ti % 4, ti // 4:ti // 4 + 1],
                                     memt[:, ti, :],
                                     start=(ti == 0), stop=(ti == NT - 1))
                cv_sb = srowp.tile([1, ENC], f32, tag="cvsb")
                nc.vector.tensor_copy(cv_sb[:], cv_ps[:])
                for ch in range(4):
                    tp = psT.tile([128, 1], f32, tag="pst", name=f"cvT{b}_{ch}")
                    nc.tensor.matmul(tp[:], cv_sb[0:1, ch * 128:(ch + 1) * 128],
                                     idf_sb[0:1, 0:1], is_transpose=True)
                    nc.vector.tensor_copy(callT_sb[ch][:, b:b + 1], tp[:])

            load_t(0, sts[0])
            load_t(1, sts[1])
            for b in range(PB):
                if b + 2 < PB and (b + 2) not in sts:
                    sts[b + 2] = load_a(b + 2)
                    load_t(b + 2, sts[b + 2])
                if b == 0 and b + 3 < PB:
                    sts[b + 3] = load_a(b + 3)
                    load_t(b + 3, sts[b + 3])
                MARK(f"a{b}")
                stage_a(b, sts[b], prev_st=sts.get(b - 1))
                if b >= 1:
                    MARK(f"b{b-1}")
                    stage_b(b - 1, sts[b - 1])
                    del sts[b - 1]
            MARK(f"b{PB-1}")
            stage_b(PB - 1, sts[PB - 1])
            MARK("epilogue")

            # ---- final: context = Call @ Wm + sig_row^T * bm ----
            ctx_ps = psM.tile([PB, U], f32, tag="misc")
            for ch in range(4):
                nc.tensor.matmul(ctx_ps[:], callT_sb[ch][:], wm_sb[ch][:],
                                 start=(ch == 0), stop=False)
            nc.tensor.matmul(ctx_ps[:], sig_row[:], bm_sb[:],
                             start=False, stop=True)
            ctx_sb = consts.tile([PB, U], f32, tag="ctx")
            nc.vector.tensor_copy(ctx_sb[:], ctx_ps[:])
            nc.sync.dma_start(out=out_d.ap(), in_=ctx_sb[:])

          for _rep in range(repeat):
              _body()
    nc.compile()
    return nc


def _host_prep(inputs):
    """Fold weights on host (weight-only transforms) and shard per core."""
    f32 = np.float32
    Wq = np.asarray(inputs["Wq"], f32)
    bq = np.asarray(inputs["bq"], f32)
    Wm = np.asarray(inputs["Wm"], f32)
    bm = np.asarray(inputs["bm"], f32)
    Wl = np.asarray(inputs["Wl"], f32)
    bl = np.asarray(inputs["bl"], f32)
    conv_w = np.asarray(inputs["conv_w"], f32)
    conv_b = np.asarray(inputs["conv_b"], f32)
    We = np.asarray(inputs["We"], f32)
    be = np.asarray(inputs["be"], f32)
    v_a = np.asarray(inputs["v_a"], f32)

    WmWe = (Wm @ We).astype(f32)
    WqWe = (Wq @ We).astype(f32)
    WlWe = (Wl @ We).astype(f32)
    G = np.einsum("ck,cu->ku", conv_w[:, 0, :], WlWe).astype(f32)
    c0 = ((bq + bm + bl) @ We + be + conv_b @ WlWe).astype(f32)

    query = np.asarray(inputs["query"], f32)
    state = np.asarray(inputs["state"], f32)
    memory = np.ascontiguousarray(np.asarray(inputs["memory"], f32))

    spad = np.zeros((B, T + 2 * PAD), f32)
    spad[:, PAD:PAD + T] = state
    q1 = np.ascontiguousarray(query[:, 1, :])

    ident = np.eye(128, dtype=f32)
    # wmwe8[kc, p, j, u] = WmWe[kc*256 + j*128 + p, u]
    wmwe8 = np.ascontiguousarray(
        WmWe.astype(E4).reshape(2, 2, 128, U).transpose(0, 2, 1, 3))
    wqwe8 = np.ascontiguousarray(
        WqWe.astype(E4).reshape(4, 2, 128, U).transpose(0, 2, 1, 3))
    g8 = np.zeros((32, U), E4)
    g8[:K] = G.astype(E4)
    g8 = np.ascontiguousarray(g8.reshape(2, 16, U).transpose(1, 0, 2))
    va8 = np.zeros((128, 4, 2, 32), E4)
    vat = v_a.astype(E4).reshape(2, 128).transpose(1, 0)  # [p, j]
    for tb in range(4):
        va8[:, tb, :, tb] = vat
    # blobb: idb | c0 row | mask4 col (rows 0-3) | wm — shared
    blobb = np.zeros((128, 1409), BF16)
    blobb[:, 0:128] = ident.astype(BF16)
    blobb[0, 128:384] = c0.astype(BF16)
    blobb[0:4, 384] = 1.0
    # per-partition flattened blob8 pieces (q1t added per core)
    wqwe_f = wqwe8.transpose(1, 0, 2, 3).reshape(128, 2048)
    wmwe_f = wmwe8.transpose(1, 0, 2, 3).reshape(128, 1024)
    va_f = va8.reshape(128, 256)
    g_f = np.zeros((128, 512), E4)
    g_f[0:16] = g8.reshape(16, 512)
    blobb[:, 385:1409] = (Wm.astype(BF16).reshape(4, 128, U)
                          .transpose(1, 0, 2).reshape(128, 1024))

    in_maps = []
    for c in range(N_CORES):
        sl = slice(c * PB, (c + 1) * PB)
        m = {"blobb": blobb}
        mb = memory[sl]  # [PB, T, ENC] f32
        # memE[b, p, kc, j, t] = e4(mem[b, t, kc*256 + j*128 + p])
        m8 = np.ascontiguousarray(mb.transpose(0, 2, 1)).astype(E4)  # [PB,ENC,T]
        m["meme"] = np.ascontiguousarray(
            m8.reshape(PB, 2, 2, 128, T).transpose(0, 3, 1, 2, 4))
        # memT[b, p, ti, e] = e3(mem[b, ti*128 + p, e])
        m["memt"] = np.ascontiguousarray(
            mb.astype(E3).reshape(PB, NT, 128, ENC).transpose(0, 2, 1, 3))
        # shifted8[b, p, j, t] = e4(spad[b, t + j*16 + p]); row (15,1) = 0
        sp8 = spad[sl].astype(E4)
        sh8 = np.zeros((PB, 32, T), E4)
        for k in range(K):
            sh8[:, k, :] = sp8[:, k:k + T]
        m["sh8"] = np.ascontiguousarray(
            sh8.reshape(PB, 2, 16, T).transpose(0, 2, 1, 3))
        # blob8: wqwe | wmwe | q1t | va | g
        q1t = np.zeros((128, 4, 2, 16), E4)
        q1t[:, :, :, 0:PB] = (q1[sl].astype(E4)
                              .reshape(PB, 4, 2, 128).transpose(3, 1, 2, 0))
        blob8 = np.zeros((128, 3968), E4)
        blob8[:, 0:1024] = wmwe_f
        blob8[:, 1024:1152] = q1t.reshape(128, 128)
        blob8[:, 1152:1408] = va_f
        blob8[:, 1408:1920] = g_f
        blob8[:, 1920:3968] = wqwe_f
        m["blob8"] = blob8
        # blobf: stateT | idf | bm row
        a = state[sl].reshape(PB, 4, 4, 128)  # [b, tb, c4, p]
        statet = a.transpose(3, 0, 2, 1).reshape(128, PB * NT)
        blobf = np.zeros((128, 512), f32)
        blobf[:, 0:128] = statet
        blobf[:, 128:256] = ident
        blobf[0, 256:512] = bm
        m["blobf"] = blobf
        in_maps.append(m)
    return in_maps


def kernel(**inputs) -> np.ndarray:
    global LAST_RESULTS
    from concourse import bass_utils

    if "nc" not in _BUILT:
        _BUILT["nc"] = _build_nc()
    nc = _BUILT["nc"]

    in_maps = _host_prep(inputs)
    res = bass_utils.run_bass_kernel_spmd(
        nc, in_maps, core_ids=list(range(N_CORES)), trace=TRACE)
    LAST_RESULTS = res
    out = np.concatenate([res.results[c]["out"] for c in range(N_CORES)], axis=0)
    return out.reshape(B, 1, U).astype(np.float32)
